# revision 7
# baseline (speedup 1.0000x reference)
"""FlowNetC correlation (max_disp=20, stride2=2) Trainium2 Bass kernel.

Full inputs: input1, input2 [8, 256, 64, 128] f32.
Output: [8, 441, 64, 128] f32 where
  out[b, dj*21+di, y, x] = mean_c in1[b,c,y,x] * in2[b,c, y+2dj-20, x+2di-20]
(zero-filled where the shifted index is out of bounds).

Sharding: pure data parallelism, one batch element per NeuronCore (8 cores).

Per-core algorithm: displacements are stride-2, so y/x parity is preserved ->
4 independent phase sub-problems, each a unit-stride +-10 correlation on a
[256, 32, 64] image. Row-correlations are 21-diagonal bands of 64x64 Gram
matrices over C=256, computed on TensorE in fp32r as [m=128 (2 rows x 64 x),
n=256 (4 rows x 64 x2)] blocks. Band-diagonal extraction cannot be expressed
as an on-chip access pattern (SBUF APs cannot encode per-partition offsets),
so Gram blocks are dumped to DRAM scratch and re-read with a skewed
(diagonal) flat-DRAM access pattern, which is legal. TensorE transposes then
put channels on partitions; a VectorE scaled copy interleaves the two
x-parities and applies the 1/256 mean; rows store with 512B-contiguous runs.
"""

import numpy as np

import concourse.bass as bass
import concourse.mybir as mybir
from concourse import bacc
from concourse.bass_utils import run_bass_kernel_spmd
from concourse.masks import make_identity
from concourse.tile import TileContext

B, C, H, W = 8, 256, 64, 128
DS, DR = 21, 10  # displacements per axis, radius
HH, XW = H // 2, W // 2  # per-phase dims: 32 rows, 64 cols
NV = 4  # in2 rows per Gram block (n = NV*XW = 256 -> fp32r full rate)
NCH = DS * DS  # 441 output channels
CHP = 448  # padded channel dim (4 transpose chunks of 112)
GROW = NV * XW  # 256: free width of one Gram block
GBLK = 128 * GROW  # flat elems of one dumped block
GPAD = 16  # flat margin; skew reads reach 10 elems outside a row section


def _vgroups():
    """Per phase: (v0, yyA, pairs) with pairs of consecutive in1 rows."""
    out = []
    for v0 in range(0, HH, NV):
        yyA = max(0, v0 - DR)
        yyB = min(HH - 1, v0 + NV - 1 + DR)
        n = yyB - yyA + 1
        assert n % 2 == 0
        pairs = [(yyA + 2 * i, yyA + 2 * i + 1) for i in range(n // 2)]
        out.append((v0, yyA, pairs))
    return out


def build_nc():
    nc = bacc.Bacc("TRN2", target_bir_lowering=False, debug=False, num_devices=1)
    in1 = nc.dram_tensor("in1", [C, H, W], mybir.dt.float32, kind="ExternalInput")
    in2 = nc.dram_tensor("in2", [C, H, W], mybir.dt.float32, kind="ExternalInput")
    out = nc.dram_tensor("out", [NCH, H, W], mybir.dt.float32, kind="ExternalOutput")
    out_t = out.ap().tensor

    vgroups = _vgroups()
    FREE = 2 * HH * W  # 8192: free size of each py-packed input tile

    with TileContext(nc) as tc:
        with (
            tc.tile_pool(name="persist", bufs=1) as persist,
            tc.tile_pool(name="gstage", bufs=4) as gstage,
            tc.tile_pool(name="band", bufs=3) as bandp,
            tc.tile_pool(name="outp", bufs=4) as outp,
            tc.tile_pool(name="psum_g", bufs=3, space="PSUM") as psg,
            tc.tile_pool(name="psum_t", bufs=4, space="PSUM") as pst,
            tc.tile_pool(name="gdump", bufs=170, space="DRAM") as gdump,
        ):
            # ---- load inputs y-parity-packed: per py a tile [ci=128, co=2, yy=32, x=128]
            # (c = co*128 + ci, y = 2*yy + py). In this layout a (row-pair, xx)
            # matmul operand is a single stride-2 progression: row step 128 = 64*2.
            in_sb = {}
            for name, src in (("i1", in1), ("i2", in2)):
                for py in range(2):
                    t = persist.tile([128, 2, HH, W], mybir.dt.float32r, name=f"{name}p{py}")
                    for co in range(2):
                        nc.sync.dma_start(
                            t[:, co],
                            bass.AP(
                                tensor=src.ap().tensor,
                                offset=co * 128 * (H * W) + py * W,
                                ap=[[H * W, 128], [2 * W, HH], [1, W]],
                            ).bitcast(mybir.dt.float32r),
                        )
                    in_sb[(name, py)] = t

            ident = persist.tile([64, 64], mybir.dt.float32)
            make_identity(nc, ident[:])

            def operand(t, co, yy0, px, nrows):
                """fp32r matmul operand [128, nrows*64]: partitions ci; the
                (row, xx) pairs of nrows consecutive packed rows form a single
                stride-2 progression."""
                off = t.offset + co * (HH * W) + yy0 * W + px
                return bass.AP(
                    tensor=t.tensor, offset=off, ap=[[FREE, 128], [2, nrows * XW]]
                )

            for py in range(2):
                gtiles = {}
                # 1) Gram blocks + dump (both x-parities)
                for px in range(2):
                    for v0, yyA, pairs in vgroups:
                        for pi, (yy1, _yy2) in enumerate(pairs):
                            pg = psg.tile([128, GROW], mybir.dt.float32)
                            for co in range(2):
                                nc.tensor.matmul(
                                    pg[:],
                                    operand(in_sb[("i1", py)], co, yy1, px, 2),
                                    operand(in_sb[("i2", py)], co, v0, px, NV),
                                    start=(co == 0),
                                    stop=(co == 1),
                                )
                            gt = gstage.tile([128, GROW], mybir.dt.float32)
                            nc.scalar.copy(gt[:], pg[:])
                            dt_ = gdump.tile([1, GBLK + 2 * GPAD], mybir.dt.float32)
                            nc.sync.dma_start(
                                bass.AP(
                                    tensor=dt_.tensor,
                                    offset=dt_.offset + GPAD,
                                    ap=[[GROW, 128], [1, GROW]],
                                ),
                                gt[:],
                            )
                            gtiles[(px, v0, pi)] = dt_

                # 2) per output row-pair index yy: extract, transpose, store
                for yy in range(HH):
                    ots = [outp.tile([112, W], mybir.dt.float32, tag=f"o{t}", name=f"ot{t}") for t in range(4)]
                    for px in range(2):
                        byy = bandp.tile([64, CHP], mybir.dt.float32)
                        nc.gpsimd.memset(byy[:], 0.0)
                        # skew-reads, batching consecutive dj in one Gram block
                        dj = 0
                        while dj < DS:
                            vv = yy + dj - DR
                            if not (0 <= vv < HH):
                                dj += 1
                                continue
                            v0 = (vv // NV) * NV
                            yyA = max(0, v0 - DR)
                            pi, yysel = (yy - yyA) // 2, (yy - yyA) % 2
                            sect0 = vv % NV
                            n = 1
                            while dj + n < DS and vv + n < HH and (vv + n) // NV == vv // NV:
                                n += 1
                            dt_ = gtiles[(px, v0, pi)]
                            src = bass.AP(
                                tensor=dt_.tensor,
                                offset=dt_.offset + GPAD + yysel * 64 * GROW + sect0 * XW - DR,
                                ap=[[GROW + 1, 64], [XW, n], [1, DS]],
                            )
                            dst = bass.AP(
                                tensor=byy.tensor,
                                offset=byy.offset + dj * DS,
                                ap=[[CHP, 64], [DS, n], [1, DS]],
                            )
                            nc.sync.dma_start(dst, src)
                            dj += n
                        # zero x-edge triangles: (xx, dj, di) invalid unless
                        # 0 <= xx + di - 10 < 64
                        nc.gpsimd.affine_select(
                            out=byy[:, :NCH],
                            in_=byy[:, :NCH],
                            compare_op=mybir.AluOpType.is_ge,
                            fill=0.0,
                            base=-DR,
                            pattern=[[0, DS], [1, DS]],
                            channel_multiplier=1,
                        )
                        nc.gpsimd.affine_select(
                            out=byy[:, :NCH],
                            in_=byy[:, :NCH],
                            compare_op=mybir.AluOpType.is_ge,
                            fill=0.0,
                            base=DR + (XW - 1),
                            pattern=[[0, DS], [-1, DS]],
                            channel_multiplier=-1,
                        )
                        for t in range(4):
                            nch = 112 if t < 3 else NCH - 336
                            pt = pst.tile([112, 64], mybir.dt.float32)
                            nc.tensor.transpose(
                                pt[:], byy[:, 112 * t : 112 * (t + 1)], ident[:]
                            )
                            dstv = bass.AP(
                                tensor=ots[t].tensor,
                                offset=ots[t].offset + px,
                                ap=[[W, nch], [2, XW]],
                            )
                            nc.vector.tensor_scalar_mul(dstv, pt[:nch, :], 1.0 / C)
                    for t in range(4):
                        nch = 112 if t < 3 else NCH - 336
                        nc.sync.dma_start(
                            bass.AP(
                                tensor=out_t,
                                offset=(112 * t) * (H * W) + (2 * yy + py) * W,
                                ap=[[H * W, nch], [1, W]],
                            ),
                            ots[t][:nch, :],
                        )

    nc.compile()
    return nc


_NC_CACHE = None


def kernel(input1: np.ndarray, input2: np.ndarray) -> np.ndarray:
    global _NC_CACHE
    input1 = np.ascontiguousarray(input1, dtype=np.float32)
    input2 = np.ascontiguousarray(input2, dtype=np.float32)
    assert input1.shape == (B, C, H, W), input1.shape
    if _NC_CACHE is None:
        _NC_CACHE = build_nc()
    nc = _NC_CACHE
    in_maps = [dict(in1=input1[b], in2=input2[b]) for b in range(B)]
    res = run_bass_kernel_spmd(nc, in_maps, core_ids=list(range(B)))
    return np.stack([r["out"] for r in res.results], axis=0)


if __name__ == "__main__":
    rng = np.random.default_rng(0)
    i1 = rng.standard_normal((B, C, H, W), dtype=np.float32)
    i2 = rng.standard_normal((B, C, H, W), dtype=np.float32)
    o = kernel(i1, i2)
    print("out", o.shape, o.dtype, float(np.abs(o).max()))
